# revision 1
# baseline (speedup 1.0000x reference)
"""Bar-level attention Trainium2 kernel (8 NeuronCores, head-parallel).

Contract: kernel(**inputs) takes the FULL inputs from setup_inputs() and
returns the FULL [1, 2048, 512] float32 output.

Strategy (one head per core, 8 heads / 8 cores):
  - Host: transpose hidden -> XT [512, 2048]; slice + transpose per-head
    weights; fold the 1/sqrt(dh) score scale into Wq/bq; compute
    g = sigmoid(gate[h]) on host and ship as replicated [128,1] columns.
  - Device (per core, all fp32):
      XT -> Q^T, K^T [64, 2048] and V [2048, 65] (col 64 = ones).
      For each 1024-wide query half and each 128-row key chunk:
        S^T = K_chunk @ Q^T  (keys on partitions, queries on free axis)
        E = exp(S^T)         (no max subtraction: scores ~ N(0,1))
        global unnorm AV  += V_chunk~.T @ E        -> [65, 1024] PSUM
        local  unnorm AV  += per-bar diagonal-block matmuls (bar_positions
                             are sorted -> blocks are contiguous; block
                             spans are baked in at build time)
        Row 64 of each AV accumulator is the softmax denominator (ones col).
      Final: project both AV results through Wo_h slice, rescale rows by
      g/l_local and (1-g)/l_global, add -> partial output [2048, 512].
  - Host: sum the 8 partial outputs (output projection is sharded over the
    contraction dim) + bo -> [1, 2048, 512].

The global-attention additive bias in the reference is per-query (constant
across keys), and softmax is shift-invariant per row, so it drops out
exactly; global attention is plain dense softmax attention.
"""

import numpy as np

S = 2048
D = 512
H = 8
DH = 64
SCALE = 1.0 / np.sqrt(DH)
NCHUNK = S // 128      # 16 key chunks of 128
NHALF = 2              # query halves of 1024
QHALF = S // NHALF


def _legalize_waits(nc, mybir):
    """This walrus codegen accepts at most ONE sync wait per instruction.
    Split any instruction carrying N>1 waits into N-1 preceding single-wait
    NoOps on the same engine (waits execute in order on the sequencer)."""
    ctr = 0
    for f in nc.m.functions:
        for b in f.blocks:
            insts = b.instructions
            if not any(i.sync_info and len(i.sync_info.on_wait) > 1 for i in insts):
                continue
            new = []
            for ins in insts:
                si = ins.sync_info
                if si is not None and len(si.on_wait) > 1:
                    waits = list(si.on_wait)
                    for w in waits[:-1]:
                        ctr += 1
                        nop = mybir.InstNoOp(name=f"waitsplit-{ctr}", engine=ins.engine)
                        nop.sync_info = mybir.SyncInfo(on_wait=[w], on_update=[])
                        new.append(nop)
                    ins.sync_info = mybir.SyncInfo(
                        on_wait=[waits[-1]], on_update=list(si.on_update)
                    )
                new.append(ins)
            insts.clear()
            insts.extend(new)
    return ctr


def _bar_bounds(bp):
    """bp: sorted int array [S] -> list of (start, end) per bar."""
    change = np.nonzero(np.diff(bp))[0] + 1
    starts = np.concatenate([[0], change])
    ends = np.concatenate([change, [len(bp)]])
    return list(zip(starts.tolist(), ends.tolist()))


def _build(bars):
    import concourse.bass as bass
    import concourse.tile as tile
    import concourse.mybir as mybir

    dt = mybir.dt
    AF = mybir.ActivationFunctionType
    OP = mybir.AluOpType
    f32 = dt.float32
    f32r = dt.float32r

    def F(ap):
        # view a float32r tile as plain fp32 (for the small local-AV matmuls
        # and DVE ops; f32r tiles hold rounded fp32 bits)
        return ap.bitcast(f32)

    nc = bass.Bass()
    xt_d = nc.dram_tensor("xt", [D, S], f32r, kind="ExternalInput")
    # wpack: 4 chunks of [128, 192]: cols 0:64 WqT(scaled), 64:128 WkT, 128:192 WvT
    wpack_d = nc.dram_tensor("wpack", [D, 192], f32r, kind="ExternalInput")
    wot_d = nc.dram_tensor("wot", [DH, D], f32r, kind="ExternalInput")
    # smalls [128, 8]: col0 bq/8 (rows 0:64), col1 bk, col2 bv, col3 g,
    # col4 1-g, col5 ones
    smalls_d = nc.dram_tensor("smalls", [128, 8], f32, kind="ExternalInput")
    zeros_d = nc.dram_tensor("zeros", [128, 512], f32r, kind="ExternalInput")
    # mask bands: chunk c occupies cols [c*512, c*512+w_c); m[kk, j] = 1 iff
    # bar(c*128+kk) == bar(blo_c + j)
    mask_d = nc.dram_tensor("maskband", [128, NCHUNK * 512], f32, kind="ExternalInput")
    out_d = nc.dram_tensor("out_partial", [S, D], f32, kind="ExternalOutput")

    # per-chunk global band [blo_c, bhi_c): union of bars intersecting chunk
    band = []
    for c in range(NCHUNK):
        klo, khi = c * 128, (c + 1) * 128
        bs = [b for b in bars if b[1] > klo and b[0] < khi]
        band.append((bs[0][0], bs[-1][1]))
        assert band[-1][1] - band[-1][0] <= 512

    with tile.TileContext(nc, pool_alloc_mode="queue") as tc:
        with (
            tc.tile_pool(name="persist", bufs=1) as p_keep,
            tc.tile_pool(name="outbuf", bufs=1) as p_out,
        ):
            qt = p_keep.tile([DH, S], f32r, tag="qt")
            kt = p_keep.tile([DH, S], f32r, tag="kt")
            zeros = p_keep.tile([128, 512], f32r, tag="zeros")
            vt = [p_keep.tile([128, DH + 1], f32r, tag=f"vt{c}", name=f"vt{c}") for c in range(NCHUNK)]
            smalls = p_keep.tile([128, 8], f32, tag="smalls")
            wot = p_keep.tile([DH, D], f32r, tag="wot")
            maskt = p_keep.tile([128, NCHUNK * 512], f32, tag="maskt")
            outbuf = p_out.tile([128, NCHUNK * D], f32, tag="outbuf")

            # ---------------- projections ----------------
            with (
                tc.tile_pool(name="inp", bufs=1) as p_in,
                tc.tile_pool(name="pj", bufs=2, space="PSUM") as p_pj,
                tc.tile_pool(name="pv", bufs=2, space="PSUM") as p_pv,
            ):
                xts = [p_in.tile([128, S], f32r, tag=f"xt{i}", name=f"xts{i}") for i in range(4)]
                wps = [p_in.tile([128, 192], f32r, tag=f"wp{i}", name=f"wps{i}") for i in range(4)]
                nc.sync.dma_start(smalls[:], smalls_d[:])
                nc.sync.dma_start(zeros[:], zeros_d[:])
                for i in range(4):
                    nc.sync.dma_start(
                        wps[i][:], wpack_d[i * 128 : (i + 1) * 128, :]
                    )
                # two column panels per tile: the first QT/KT/V matmul groups
                # unlock after ~2MB instead of the full 4MB. The 4MB mask
                # band is deferred: the shared SDMA engines serialize
                # transfers, and the mask isn't consumed until the first
                # local-AV (~25us in).
                for n in range(2):
                    for i in range(4):
                        nc.sync.dma_start(
                            xts[i][:, n * 1024 : (n + 1) * 1024],
                            xt_d[i * 128 : (i + 1) * 128, n * 1024 : (n + 1) * 1024],
                        )
                # mask on the same (sync) queue: a gpsimd-issued DMA would
                # start immediately (Pool engine idle) and hog the shared
                # SDMA engines ahead of the XT panels
                nc.sync.dma_start(maskt[:], mask_d[:])
                nc.sync.dma_start(wot[:], wot_d[:])

                # Q^T and K^T: [64, 2048] in 1024-halves through 2 psum bufs
                for which, dest, wcol, bcol in ((0, qt, 0, 0), (1, kt, 64, 1)):
                    for hq in range(NHALF):
                        ps = p_pj.tile([DH, QHALF], f32, tag="pj")
                        for n in range(QHALF // 512):
                            for kc in range(4):
                                nc.tensor.matmul(
                                    ps[:, n * 512 : (n + 1) * 512],
                                    (wps[kc][:, wcol : wcol + 64]),
                                    (xts[kc][
                                        :,
                                        hq * QHALF + n * 512 : hq * QHALF + (n + 1) * 512,
                                    ]),
                                    start=(kc == 0),
                                    stop=(kc == 3),
                                )
                        nc.scalar.activation(
                            dest[:, hq * QHALF : (hq + 1) * QHALF],
                            ps[:],
                            AF.Identity,
                            bias=smalls[0:DH, bcol : bcol + 1],
                        )

                # V in natural [k, dh] layout, chunk by chunk; col 64 = 1.0
                for c in range(NCHUNK):
                    ps = p_pv.tile([128, DH], f32, tag="pv")
                    for kc in range(4):
                        nc.tensor.matmul(
                            ps[:],
                            (xts[kc][:, c * 128 : (c + 1) * 128]),
                            (wps[kc][:, 128:192]),
                            start=(kc == 0),
                            stop=(kc == 3),
                        )
                    nc.scalar.activation(
                        vt[c][:, 0:DH],
                        ps[:],
                        AF.Identity,
                        bias=smalls[:, 2:3],
                    )
                    nc.scalar.copy(vt[c][:, DH : DH + 1], smalls[:, 5:6])

            # ---------------- attention ----------------
            # per-(half, chunk) bar pieces baked from bar_positions
            ogs = []
            ols = []
            with tc.tile_pool(name="avout", bufs=1) as p_av:
                l2l = p_av.tile([128, NCHUNK], f32r, tag="l2l")
                l2g = p_av.tile([128, NCHUNK], f32r, tag="l2g")
                r2l = p_av.tile([128, NCHUNK], f32, tag="r2l")
                r2g = p_av.tile([128, NCHUNK], f32, tag="r2g")
                with (
                    tc.tile_pool(name="ps", bufs=2, space="PSUM") as p_s,
                    tc.tile_pool(name="pog", bufs=1, space="PSUM") as p_og,
                    tc.tile_pool(name="pol", bufs=1, space="PSUM") as p_ol,
                    tc.tile_pool(name="pe", bufs=3) as p_e,
                    tc.tile_pool(name="pel", bufs=2) as p_el,
                ):
                  for hq in range(NHALF):
                    og_sb = p_av.tile([DH + 1, QHALF], f32r, tag=f"og{hq}", name=f"ogsb{hq}")
                    ol_sb = p_av.tile([DH + 1, QHALF], f32r, tag=f"ol{hq}", name=f"olsb{hq}")
                    ogs.append(og_sb)
                    ols.append(ol_sb)
                    if True:
                        og = p_og.tile([DH + 1, QHALF], f32, tag="og", name=f"og{hq}")
                        ol = p_ol.tile([DH + 1, QHALF], f32, tag="ol", name=f"ol{hq}")
                        # HW: start=True clears has_written for the WHOLE
                        # bank, so interleaved per-region accumulation groups
                        # corrupt each other. Zero-init ol once with a full
                        # width start=True matmul against zeros, then every
                        # local piece accumulates with start=False.
                        for n in range(QHALF // 512):
                            nc.tensor.matmul(
                                ol[:, n * 512 : (n + 1) * 512],
                                vt[0][:],
                                zeros[:],
                                start=True,
                                stop=False,
                                skip_group_check=True,
                            )
                        for c in range(NCHUNK):
                            sc = p_s.tile([128, QHALF], f32, tag="s")
                            for n in range(QHALF // 512):
                                nc.tensor.matmul(
                                    sc[:, n * 512 : (n + 1) * 512],
                                    (kt[:, c * 128 : (c + 1) * 128]),
                                    (qt[
                                        :,
                                        hq * QHALF + n * 512 : hq * QHALF + (n + 1) * 512,
                                    ]),
                                    start=True,
                                    stop=True,
                                )
                            ec = p_e.tile([128, QHALF], f32r, tag="e")
                            nc.scalar.activation(ec[:], sc[:], AF.Exp)
                            # global AV accumulation
                            for n in range(QHALF // 512):
                                nc.tensor.matmul(
                                    og[:, n * 512 : (n + 1) * 512],
                                    (vt[c][:]),
                                    (ec[:, n * 512 : (n + 1) * 512]),
                                    start=(c == 0),
                                    stop=(c == NCHUNK - 1),
                                )
                            # local AV: masked band of E (bars are contiguous
                            # diagonal blocks); matmul base partitions must be
                            # 0/32/64, so zero-pad a full-128-row band copy.
                            klo, khi = c * 128, (c + 1) * 128
                            qlo, qhi = hq * QHALF, (hq + 1) * QHALF
                            pieces = []  # (qs, qe, rlo, rhi, start, stop)
                            for (s_b, e_b) in bars:
                                if e_b <= klo or s_b >= khi:
                                    continue
                                qs = max(s_b, qlo)
                                qe = min(e_b, qhi)
                                if qs >= qe:
                                    continue
                                pieces.append(
                                    (
                                        qs,
                                        qe,
                                        max(s_b, klo) - klo,
                                        min(e_b, khi) - klo,
                                        s_b >= klo,
                                        e_b <= khi,
                                    )
                                )
                            if pieces:
                                blo, bhi = band[c]
                                hs = pieces[0][0]   # half-clipped band start
                                he = pieces[-1][1]
                                w = he - hs
                                el = p_el.tile([128, 512], f32, tag="el", name="el")
                                nc.vector.tensor_mul(
                                    el[:, 0:w],
                                    F(ec[:, hs - qlo : he - qlo]),
                                    maskt[:, c * 512 + (hs - blo) : c * 512 + (he - blo)],
                                )
                                # matmul runs: merge adjacent pieces with same
                                # flags, split at 512-col psum bank boundaries
                                runs = []
                                for (qs, qe, _, _, st, sp) in pieces:
                                    if runs and runs[-1][2] == st and runs[-1][3] == sp and runs[-1][1] == qs:
                                        runs[-1][1] = qe
                                    else:
                                        runs.append([qs, qe, st, sp])
                                for (qs, qe, st, sp) in runs:
                                    a = qs
                                    while a < qe:
                                        b_ = min(qe, ((a - qlo) // 512 + 1) * 512 + qlo)
                                        nc.tensor.matmul(
                                            ol[:, a - qlo : b_ - qlo],
                                            F(vt[c][:]),
                                            el[:, a - hs : b_ - hs],
                                            start=False,
                                            stop=False,
                                            skip_group_check=True,
                                        )
                                        a = b_
                        # close the ol accumulation group (adds zeros)
                        for n in range(QHALF // 512):
                            nc.tensor.matmul(
                                ol[:, n * 512 : (n + 1) * 512],
                                vt[0][:],
                                zeros[:],
                                start=False,
                                stop=True,
                                skip_group_check=True,
                            )
                        nc.scalar.copy(og_sb[:], og[:])
                        nc.scalar.copy(ol_sb[:], ol[:])
                    # denominator rows -> [128, 8] reshape, natural order:
                    # l2[p, hq*8+jj] = l_half[p*8+jj]; issued per half so
                    # half0's transfers hide under half1's attention
                    j0 = hq * (NCHUNK // NHALF)
                    nc.sync.dma_start(
                        l2l[:, j0 : j0 + NCHUNK // NHALF], ol_sb[DH : DH + 1, :]
                    )
                    nc.sync.dma_start(
                        l2g[:, j0 : j0 + NCHUNK // NHALF], og_sb[DH : DH + 1, :]
                    )

                # ---------------- denominators + recip ----------------
                if True:
                    nc.vector.reciprocal(r2l[:], F(l2l[:]))
                    nc.vector.reciprocal(r2g[:], F(l2g[:]))
                    # fold gate: r_l *= g, r_g *= (1-g)
                    nc.vector.tensor_scalar_mul(r2l[:], r2l[:], smalls[:, 3:4])
                    nc.vector.tensor_scalar_mul(r2g[:], r2g[:], smalls[:, 4:5])

                    # ---------------- output projection + combine ----------
                    with (
                        tc.tile_pool(name="pp", bufs=4, space="PSUM") as p_pp,
                        tc.tile_pool(name="t1", bufs=2) as p_t1,
                    ):
                        for j in range(NCHUNK):
                            hq = j // (NCHUNK // NHALF)
                            jj = j % (NCHUNK // NHALF)
                            # interleaved query chunk: cols jj, jj+8, ...
                            lp = p_pp.tile([128, D], f32, tag="pp")
                            nc.tensor.matmul(
                                lp[:],
                                (ols[hq][0:DH, jj : QHALF : NCHUNK // NHALF]),
                                (wot[:]),
                                start=True,
                                stop=True,
                            )
                            gp = p_pp.tile([128, D], f32, tag="pp")
                            nc.tensor.matmul(
                                gp[:],
                                (ogs[hq][0:DH, jj : QHALF : NCHUNK // NHALF]),
                                (wot[:]),
                                start=True,
                                stop=True,
                            )
                            t1 = p_t1.tile([128, D], f32, tag="t1")
                            # t1 = lp * r_l[q]  (per-partition scale)
                            nc.vector.tensor_scalar_mul(
                                t1[:], lp[:], r2l[:, j : j + 1]
                            )
                            # out = gp * r_g[q] + t1  (DVE fused)
                            nc.vector.scalar_tensor_tensor(
                                outbuf[:, j * D : (j + 1) * D],
                                gp[:],
                                r2g[:, j : j + 1],
                                t1[:],
                                OP.mult,
                                OP.add,
                            )
                            grp = {3: (0, 4), 7: (4, 4), 11: (8, 4),
                                   13: (12, 2), 14: (14, 1), 15: (15, 1)}.get(j)
                            if grp:
                                c0, ng = grp
                                hq_ = c0 // (NCHUNK // NHALF)
                                jj0 = c0 % (NCHUNK // NHALF)
                                dst = out_d[
                                    hq_ * QHALF : (hq_ + 1) * QHALF, :
                                ].rearrange("(p j) c -> p j c", j=NCHUNK // NHALF)[
                                    :, jj0 : jj0 + ng, :
                                ]
                                srcb = outbuf[
                                    :, c0 * D : (c0 + ng) * D
                                ].rearrange("p (j c) -> p j c", j=ng)
                                nc.sync.dma_start(dst, srcb)

    _legalize_waits(nc, mybir)
    return nc


_CACHE = {}


def _get_built(bar_key, bars):
    if bar_key not in _CACHE:
        _CACHE[bar_key] = _build(bars)
    return _CACHE[bar_key]


def _np_reference(hidden_states, bar_positions, attention_mask, Wq, bq, Wk, bk,
                  Wv, bv, Wo, bo, bar_emb, gate):
    """Plain numpy fallback (only used if inputs violate baked assumptions)."""
    B, S_, _ = hidden_states.shape
    x = hidden_states.astype(np.float64)
    q = (x @ Wq.T + bq).reshape(B, S_, H, DH).transpose(0, 2, 1, 3)
    k = (x @ Wk.T + bk).reshape(B, S_, H, DH).transpose(0, 2, 1, 3)
    v = (x @ Wv.T + bv).reshape(B, S_, H, DH).transpose(0, 2, 1, 3)
    scores = np.einsum("bhqd,bhkd->bhqk", q, k) * SCALE
    pad = attention_mask[:, None, None, :]
    bar_mask = (bar_positions[:, :, None] == bar_positions[:, None, :])[:, None]
    NEG = -np.inf

    def softmax(s):
        s = s - s.max(-1, keepdims=True)
        e = np.exp(s)
        return e / e.sum(-1, keepdims=True)

    local = softmax(np.where(bar_mask & pad, scores, NEG))
    emb = bar_emb[np.asarray(bar_positions) % bar_emb.shape[0]]
    bias = np.sum(emb * emb, axis=-1)
    glob = softmax(np.where(pad, scores + bias[:, None, :, None], NEG))
    la = np.einsum("bhqk,bhkd->bhqd", local, v)
    ga = np.einsum("bhqk,bhkd->bhqd", glob, v)
    g = 1.0 / (1.0 + np.exp(-gate))[None, :, None, None]
    comb = g * la + (1.0 - g) * ga
    out = comb.transpose(0, 2, 1, 3).reshape(B, S_, H * DH)
    return (out @ Wo.T + bo).astype(np.float32)


def kernel(**inputs):
    hidden_states = np.asarray(inputs["hidden_states"], dtype=np.float32)
    bar_positions = np.asarray(inputs["bar_positions"])
    attention_mask = np.asarray(inputs["attention_mask"])
    Wq = np.asarray(inputs["Wq"], dtype=np.float32)
    bq = np.asarray(inputs["bq"], dtype=np.float32)
    Wk = np.asarray(inputs["Wk"], dtype=np.float32)
    bk = np.asarray(inputs["bk"], dtype=np.float32)
    Wv = np.asarray(inputs["Wv"], dtype=np.float32)
    bv = np.asarray(inputs["bv"], dtype=np.float32)
    Wo = np.asarray(inputs["Wo"], dtype=np.float32)
    bo = np.asarray(inputs["bo"], dtype=np.float32)
    gate = np.asarray(inputs["gate"], dtype=np.float32)

    bp = bar_positions[0].astype(np.int64)
    if (
        hidden_states.shape != (1, S, D)
        or not bool(attention_mask.all())
        or not bool((np.diff(bp) >= 0).all())
    ):
        return _np_reference(
            hidden_states, bar_positions, attention_mask, Wq, bq, Wk, bk,
            Wv, bv, Wo, bo, np.asarray(inputs["bar_emb"], dtype=np.float32), gate,
        )

    bars = _bar_bounds(bp)
    nc = _get_built(bp.tobytes(), bars)

    # mask bands (same for every core)
    maskband = np.zeros((128, NCHUNK * 512), dtype=np.float32)
    for c in range(NCHUNK):
        klo, khi = c * 128, (c + 1) * 128
        bs = [b for b in bars if b[1] > klo and b[0] < khi]
        blo = bs[0][0]
        eq = (bp[klo:khi, None] == bp[None, blo : bs[-1][1]])
        maskband[:, c * 512 : c * 512 + eq.shape[1]] = eq.astype(np.float32)

    xt = np.ascontiguousarray(hidden_states[0].T)  # [512, 2048]
    g = 1.0 / (1.0 + np.exp(-gate.astype(np.float64)))  # sigmoid, [H]
    in_maps = []
    for h in range(H):
        sl = slice(h * DH, (h + 1) * DH)
        wpack = np.empty((D, 192), dtype=np.float32)
        wpack[:, 0:64] = Wq[sl, :].T * np.float32(SCALE)
        wpack[:, 64:128] = Wk[sl, :].T
        wpack[:, 128:192] = Wv[sl, :].T
        wot = np.ascontiguousarray(Wo[:, sl].T)  # [64, 512]
        smalls = np.zeros((128, 8), dtype=np.float32)
        smalls[0:DH, 0] = bq[sl] * np.float32(SCALE)
        smalls[0:DH, 1] = bk[sl]
        smalls[0:DH, 2] = bv[sl]
        smalls[:, 3] = np.float32(g[h])
        smalls[:, 5] = 1.0
        smalls[:, 4] = np.float32(1.0 - g[h])
        in_maps.append(
            {"xt": xt, "wpack": wpack, "wot": wot, "smalls": smalls,
             "maskband": maskband, "zeros": np.zeros((128, 512), np.float32)}
        )

    res = _run_spmd(nc, in_maps)
    out = np.zeros((S, D), dtype=np.float32)
    for h in range(H):
        out += res.results[h]["out_partial"]
    out += bo
    return out.reshape(1, S, D)


def _run_spmd(nc, in_maps, **kw):
    from concourse.bass_utils import run_bass_kernel_spmd

    return run_bass_kernel_spmd(nc, in_maps, list(range(H)), **kw)

